# revision 64
# baseline (speedup 1.0000x reference)
"""BlockGRU Trainium2 kernel.

Block-diagonal GRU cell: 8 independent blocks (block_size 256), batch 2048,
input_dim 1024. Sharded one block per NeuronCore (8 cores).

Per-core layout: gates on partitions, batch on the free dimension
(everything transposed on the host, which is free). Matmul operands are
fp16 (measured end-to-end rel-L2 error vs the fp32 reference: 2.6e-4;
fp16 halves the DMA streams and runs the PE at full rate with fast
weight load); accumulation and all elementwise math stay fp32. r/z gate
pre-activations accumulate input-projection + hidden-projection directly
in PSUM; i_n and h_n are kept separate for the r-gating. Per-partition
biases fuse into ScalarE activation ops (sigmoid/tanh) and a
scalar_tensor_tensor on VectorE; 1-z runs on the idle GPSIMD engine.
"""

import sys

if "/opt/trn_rl_repo" not in sys.path:
    sys.path.insert(0, "/opt/trn_rl_repo")

import numpy as np

INPUT_DIM = 1024
HIDDEN_DIM = 2048
NUM_BLOCKS = 8
BS = HIDDEN_DIM // NUM_BLOCKS  # 256
G3 = 3 * BS                    # 768
BATCH = 2048
CHUNKS = [512, 512, 512, 256, 256]   # batch chunks (PSUM bank = 512 fp32;
                                     # small tail chunks shorten the post-PE tail)
KX = INPUT_DIM // 128          # 8 contraction tiles on the input side
KH = BS // 128                 # 2 contraction tiles on the hidden side
ST = BS // 128                 # 2 state partition-tiles per block

_cached = None


def _build():
    import concourse.tile as tile
    import concourse.mybir as mybir
    from concourse import bacc

    f32 = mybir.dt.float32
    f16 = mybir.dt.float16
    ALU = mybir.AluOpType
    ACT = mybir.ActivationFunctionType

    nc = bacc.Bacc("TRN2", target_bir_lowering=False, debug=False, num_devices=8)

    xT = nc.dram_tensor("xT", [INPUT_DIM, BATCH], f16, kind="ExternalInput")
    wih = nc.dram_tensor("wih", [INPUT_DIM, G3], f16, kind="ExternalInput")
    whh = nc.dram_tensor("whh", [BS, G3], f16, kind="ExternalInput")
    hT = nc.dram_tensor("hT", [BS, BATCH], f16, kind="ExternalInput")
    bias = nc.dram_tensor("bias", [128, 5 * ST], f32, kind="ExternalInput")
    oT = nc.dram_tensor("oT", [BS, BATCH], f32, kind="ExternalOutput")

    with tile.TileContext(nc) as tc:
        with (
            tc.tile_pool(name="const", bufs=1) as cp,
            tc.tile_pool(name="xin", bufs=3) as xp,
            tc.tile_pool(name="hin", bufs=3) as hp,
            tc.tile_pool(name="gates", bufs=4) as gp,
            tc.tile_pool(name="outs", bufs=3) as op,
            tc.tile_pool(name="psum", bufs=1, space="PSUM") as pp,
        ):
            # PE warm-up: harmless matmuls on a zeroed tile while the prefill
            # DMA runs, so the clock ramp (cold -> full rate) completes before
            # real work arrives. Uses the p0 PSUM slot ahead of chunk 0.
            wu = cp.tile([128, 32], f16, tag="wu")
            nc.vector.memset(wu[:], 0.0)
            pdummy = pp.tile([128, 32], f32, tag="p0", name="pdummy")
            for _ in range(48):
                nc.tensor.matmul(pdummy[0:32, :], wu[:, 0:32], wu[:],
                                 start=True, stop=True)

            # --- DMA prologue. The DMA queue is serial at HBM bandwidth, so
            # emission order == arrival order == PE consumption order: the
            # x-side weights and chunk-0 columns first (bulk of PE work),
            # hidden-side weights/state + biases after (consumed at the end
            # of chunk 0's accumulation). ---
            c0w = CHUNKS[0]
            cs0 = slice(0, c0w)
            wih_sb = []
            x0_t = []
            # k-tiles load pairwise-merged via 3D access patterns: fewer
            # DMA descriptors means the serial DMA stream outpaces PE's
            # k-major consumption, so chunk 0 runs stall-free.
            for kp in range(0, KX, 2):
                wm = cp.tile([128, 2 * G3], f16, tag=f"wih{kp}", name=f"wihm{kp}")
                nc.sync.dma_start(
                    wm[:].rearrange("p (k g) -> p k g", k=2),
                    wih.ap()[kp * 128:(kp + 2) * 128, :]
                        .rearrange("(k p) g -> p k g", p=128))
                wih_sb.append(wm[:, 0:G3])
                wih_sb.append(wm[:, G3:2 * G3])
                xm = xp.tile([128, 2 * c0w], f16, tag=f"x{kp}", name=f"xm{kp}")
                nc.sync.dma_start(
                    xm[:].rearrange("p (k c) -> p k c", k=2),
                    xT.ap()[kp * 128:(kp + 2) * 128, cs0]
                        .rearrange("(k p) b -> p k b", p=128))
                x0_t.append(xm[:, 0:c0w])
                x0_t.append(xm[:, c0w:2 * c0w])
                if kp == 4:
                    bias_sb = cp.tile([128, 5 * ST], f32, tag="bias")
                    nc.sync.dma_start(bias_sb[:], bias.ap())
            brz_sb = bias_sb[:, 0:2 * ST]
            bzn_sb = bias_sb[:, 2 * ST:3 * ST]
            bin_sb = bias_sb[:, 3 * ST:4 * ST]
            bhn_sb = bias_sb[:, 4 * ST:5 * ST]
            whm = cp.tile([128, 2 * G3], f16, tag="whm")
            nc.sync.dma_start(
                whm[:].rearrange("p (k g) -> p k g", k=2),
                whh.ap().rearrange("(k p) g -> p k g", p=128))
            whh_sb = [whm[:, 0:G3], whm[:, G3:2 * G3]]
            h0m = hp.tile([128, 2 * c0w], f16, tag="h0m")
            nc.sync.dma_start(
                h0m[:].rearrange("p (k c) -> p k c", k=2),
                hT.ap()[:, cs0].rearrange("(k p) b -> p k b", p=128))
            h0_t = [h0m[:, 0:c0w], h0m[:, c0w:2 * c0w]]

            cstart = 0
            for c, cw in enumerate(CHUNKS):
                cs = slice(cstart, cstart + cw)
                cstart += cw
                if c == 0:
                    x_t, h_t = x0_t, h0_t
                else:
                    x_t = []
                    for kp in range(0, KX, 2):
                        xm2 = xp.tile([128, 2 * cw], f16, tag=f"x{kp}",
                                      name=f"xc{kp}")
                        nc.sync.dma_start(
                            xm2[:].rearrange("p (k c) -> p k c", k=2),
                            xT.ap()[kp * 128:(kp + 2) * 128, cs]
                                .rearrange("(k p) b -> p k b", p=128))
                        x_t.append(xm2[:, 0:cw])
                        x_t.append(xm2[:, cw:2 * cw])
                    hm2 = hp.tile([128, 2 * cw], f16, tag="h0m", name="hc")
                    nc.sync.dma_start(
                        hm2[:].rearrange("p (k c) -> p k c", k=2),
                        hT.ap()[:, cs].rearrange("(k p) b -> p k b", p=128))
                    h_t = [hm2[:, 0:cw], hm2[:, cw:2 * cw]]

                # PSUM accumulators. r/z gates take input-proj + hidden-proj
                # into the same bank (only their sum is needed downstream).
                p_rz = [pp.tile([128, cw], f32, tag=f"p{gt}", name=f"prz{gt}")
                        for gt in range(2 * ST)]
                p_in = [pp.tile([128, cw], f32, tag=f"p{2 * ST + t_}", name=f"pin{t_}")
                        for t_ in range(ST)]
                p_hn = [pp.tile([128, cw], f32, tag=f"p{3 * ST + t_}", name=f"phn{t_}")
                        for t_ in range(ST)]

                # Input-side first, k-major, so PE consumption tracks the DMA
                # arrival order (wih[k]/x[k] pairs).  The last x k-tile plus
                # all hidden-side matmuls form per-psum "tail groups" ordered
                # so psums complete staggered: r-gates first (sigmoids start
                # draining banks early), i_n last (shortest post-PE chain).
                def gsl(gt):
                    return slice(gt * 128, (gt + 1) * 128)

                for k in range(KX - 1):
                    for gt in range(2 * ST):
                        nc.tensor.matmul(p_rz[gt][:], wih_sb[k][:, gsl(gt)],
                                         x_t[k][:], start=(k == 0), stop=False)
                    for t_ in range(ST):
                        nc.tensor.matmul(p_in[t_][:], wih_sb[k][:, gsl(4 + t_)],
                                         x_t[k][:], start=(k == 0), stop=False)
                kl = KX - 1
                last = (c == len(CHUNKS) - 1)
                o = op.tile([128, ST * cw], f32, tag="o")

                def r_tail(t_):
                    nc.tensor.matmul(p_rz[t_][:], wih_sb[kl][:, gsl(t_)],
                                     x_t[kl][:], start=False, stop=False)
                    for k in range(KH):
                        nc.tensor.matmul(p_rz[t_][:], whh_sb[k][:, gsl(t_)],
                                         h_t[k][:], start=False, stop=(k == KH - 1))

                def hn_tail(t_):
                    for k in range(KH):
                        nc.tensor.matmul(p_hn[t_][:], whh_sb[k][:, gsl(4 + t_)],
                                         h_t[k][:], start=(k == 0), stop=(k == KH - 1))

                def in_tail(t_):
                    nc.tensor.matmul(p_in[t_][:], wih_sb[kl][:, gsl(4 + t_)],
                                     x_t[kl][:], start=False, stop=True)

                def z_tail(t_):
                    gt = ST + t_
                    nc.tensor.matmul(p_rz[gt][:], wih_sb[kl][:, gsl(gt)],
                                     x_t[kl][:], start=False, stop=False)
                    for k in range(KH):
                        nc.tensor.matmul(p_rz[gt][:], whh_sb[k][:, gsl(gt)],
                                         h_t[k][:], start=False, stop=(k == KH - 1))

                def ew_r(t_):
                    r = gp.tile([128, cw], f32, tag=f"r{t_}", name=f"r{t_}")
                    nc.scalar.activation(r[:], p_rz[t_][:], ACT.Sigmoid,
                                         bias=brz_sb[:, t_:t_ + 1])
                    a = gp.tile([128, cw], f32, tag=f"a{t_}", name=f"a{t_}")
                    nc.vector.scalar_tensor_tensor(
                        a[:], p_hn[t_][:], bhn_sb[:, t_:t_ + 1], r[:],
                        ALU.add, ALU.mult)
                    return a

                def ew_z(t_):
                    z = gp.tile([128, cw], f32, tag=f"z{t_}", name=f"z{t_}")
                    nc.scalar.activation(z[:], p_rz[ST + t_][:], ACT.Sigmoid,
                                         bias=brz_sb[:, ST + t_:ST + t_ + 1])
                    zc = gp.tile([128, cw], f32, tag=f"zc{t_}", name=f"zc{t_}")
                    nc.gpsimd.tensor_scalar(zc[:], z[:], -1.0, 1.0,
                                            ALU.mult, ALU.add)
                    return z, zc

                def ew_zh(t_, z):
                    zh = gp.tile([128, cw], f32, tag=f"zh{t_}", name=f"zh{t_}")
                    nc.vector.tensor_mul(zh[:], z[:], h_t[t_][:])
                    return zh

                def ew_tanh(t_, a):
                    b2 = gp.tile([128, cw], f32, tag=f"b{t_}", name=f"b{t_}")
                    nc.vector.tensor_add(b2[:], a[:], p_in[t_][:])
                    n_ = gp.tile([128, cw], f32, tag=f"n{t_}", name=f"n{t_}")
                    nc.scalar.activation(n_[:], b2[:], ACT.Tanh,
                                         bias=bin_sb[:, t_:t_ + 1])
                    return n_

                def ew_out(t_, n_, zc, zh):
                    e = gp.tile([128, cw], f32, tag=f"e{t_}", name=f"e{t_}")
                    nc.vector.tensor_mul(e[:], n_[:], zc[:])
                    nc.vector.tensor_add(o[:, t_ * cw:(t_ + 1) * cw], e[:],
                                         zh[:])

                if not last:
                    # staggered psum completion: r-gates first (sigmoids free
                    # banks for the next chunk), i_n last (short post chain)
                    for t_ in range(ST):
                        r_tail(t_)
                    for t_ in range(ST):
                        hn_tail(t_)
                    for t_ in range(ST):
                        z_tail(t_)
                    for t_ in range(ST):
                        in_tail(t_)
                    as_ = [ew_r(t_) for t_ in range(ST)]
                    zzc = [ew_z(t_) for t_ in range(ST)]
                    zhs = [ew_zh(t_, zzc[t_][0]) for t_ in range(ST)]
                    ns_ = [ew_tanh(t_, as_[t_]) for t_ in range(ST)]
                    for t_ in range(ST):
                        ew_out(t_, ns_[t_], zzc[t_][1], zhs[t_])
                    nc.scalar.dma_start(
                        oT.ap().rearrange("(t p) b -> p t b", p=128)[:, :, cs],
                        o[:].rearrange("p (t c) -> p t c", t=ST))
                else:
                    # final chunk: i_n psums complete before the z-gates so
                    # the b2/tanh chain runs under the last matmuls; b2 goes
                    # ahead of zh on the VectorE queue; per-tile output DMAs
                    # on the scalar and sync DGE queues.
                    for t_ in range(ST):
                        r_tail(t_)
                    for t_ in range(ST):
                        hn_tail(t_)
                    for t_ in range(ST):
                        in_tail(t_)
                    for t_ in range(ST):
                        z_tail(t_)
                    as_ = [ew_r(t_) for t_ in range(ST)]
                    zzc = [ew_z(t_) for t_ in range(ST)]
                    ns_ = [ew_tanh(t_, as_[t_]) for t_ in range(ST)]
                    zhs = [ew_zh(t_, zzc[t_][0]) for t_ in range(ST)]
                    for t_ in range(ST):
                        ew_out(t_, ns_[t_], zzc[t_][1], zhs[t_])
                        eng = nc.scalar if t_ == 0 else nc.sync
                        eng.dma_start(
                            oT.ap()[t_ * 128:(t_ + 1) * 128, cs],
                            o[:, t_ * cw:(t_ + 1) * cw])

    nc.compile()
    return nc


def _get_nc():
    global _cached
    if _cached is None:
        _cached = _build()
    return _cached


def kernel(input, hidden, W_ih, W_hh, b_ih, b_hh):
    input = np.asarray(input, dtype=np.float32)
    hidden = np.asarray(hidden, dtype=np.float32)
    W_ih = np.asarray(W_ih, dtype=np.float32)
    W_hh = np.asarray(W_hh, dtype=np.float32)
    b_ih = np.asarray(b_ih, dtype=np.float32)
    b_hh = np.asarray(b_hh, dtype=np.float32)

    nc = _get_nc()
    from concourse.bass_utils import run_bass_kernel_spmd

    xT = np.ascontiguousarray(input.T.astype(np.float16))
    in_maps = []
    for n in range(NUM_BLOCKS):
        brz_n = (b_ih[n, :2 * BS] + b_hh[n, :2 * BS]).reshape(2 * ST, 128).T
        bzn_n = -brz_n[:, ST:]
        bin_n = b_ih[n, 2 * BS:].reshape(ST, 128).T
        bhn_n = b_hh[n, 2 * BS:].reshape(ST, 128).T
        bias_n = np.concatenate([brz_n, bzn_n, bin_n, bhn_n], axis=1)
        in_maps.append({
            "xT": xT,
            "wih": np.ascontiguousarray(W_ih[n].T.astype(np.float16)),
            "whh": np.ascontiguousarray(W_hh[n].T.astype(np.float16)),
            "hT": np.ascontiguousarray(hidden[:, n * BS:(n + 1) * BS].T.astype(np.float16)),
            "bias": np.ascontiguousarray(bias_n),
        })

    res = run_bass_kernel_spmd(nc, in_maps, core_ids=list(range(NUM_BLOCKS)))
    out = np.empty((BATCH, HIDDEN_DIM), dtype=np.float32)
    for n in range(NUM_BLOCKS):
        out[:, n * BS:(n + 1) * BS] = res.results[n]["oT"].T
    return out
